# revision 7
# baseline (speedup 1.0000x reference)
"""Trainium2 Bass kernel for nn_ConfidenceAdaptiveSystem (MoE confidence routing).

Reference semantics (B=8192, D=4096, H=8192, C=2):
    t_out = relu(x @ t_w1 + t_b1) @ t_w2 + t_b2
    conf  = max(softmax(t_out, axis=1))          # conf<0.8 <=> |t0-t1| < ln4
    f_out = relu(x @ f_w1 + f_b1) @ f_w2 + f_b2
    out   = where(conf < 0.8, f_out, t_out)

Sharding: data-parallel over batch across 8 cores (1024 rows each), weights
replicated. Computed transposed ([feature, batch]); weights stationary.

Precision strategy: the routing threshold is razor-thin (min margin on
key-0 data is 1.7e-4), so the t-expert pre-activations need ~2^-14
accuracy while values only need ~1e-2. The baseline used a 3-pass bf16
split. Here: main pass in fp16 (11-bit mantissa, same 1 cyc/row on the
PE as bf16), plus ONE fp8 DoubleRow matmul (2 fp8 weights per PE cell,
K=256 per pass, ~0.5-0.6x a bf16 pass) carrying both correction terms:
ko=0 is (x_lo*2^14)@(w_hi*2^4), ko=1 is (x*2^5)@(w_lo*2^13), products at
scale 2^18 matching the main pass whose fp16 weights are pre-scaled by
2^18 (fp16 max 65504 caps the scale). All three accumulate into the SAME
PSUM bank; the 2^18 output scale is undone by pre-scaling t_w2 by 2^-18
(relu(c*x)=c*relu(x)). Residual ~2^-15. The f-expert only contributes
values: one fp16 pass (w*2^10, undone via f_w2).
Matmul1 work: 2.5-2.6 bf16-pass equivalents vs the baseline's 4.
"""

import numpy as np
import ml_dtypes

import concourse.bass as bass
import concourse.mybir as mybir
from concourse.tile import TileContext
from concourse.bass_utils import run_bass_kernel_spmd

F32 = mybir.dt.float32
BF16 = mybir.dt.bfloat16
F16 = mybir.dt.float16
FP8 = mybir.dt.float8e4
NP_FP8 = mybir.dt.np(mybir.dt.float8e4)  # ml_dtypes.float8_e4m3, max 240
LN4 = float(np.log(0.8 / 0.2))  # |t0-t1| < LN4  <=>  conf < 0.8

N_CORES = 8
B, D, H, C = 8192, 4096, 8192, 2

# product scale of the t-expert first-layer accumulation (fp16 max caps it)
SCALE_BITS = 18
S_XLO = 14   # x_lo * 2^14  in fp8
S_WHI = 4    # w_hi * 2^4   in fp8   (14+4 = 18)
S_XHI = 5    # x    * 2^5   in fp8
S_WLO = 13   # w_lo * 2^13  in fp8   (5+13 = 18)
F_SCALE = 10  # f-expert first-layer product scale


def build_nc(Bc, D_, H_):
    """Build the per-core Bass program. Bc: rows per core."""
    KT = D_ // 128   # k-tiles of matmul1
    MT = H_ // 128   # m-tiles (h dim)
    NT = Bc // 512   # n-tiles (batch dim)
    nc = bass.Bass(trn_type="TRN2")

    # ---- DRAM parameters (per core) ----
    xhi = nc.declare_dram_parameter("xhi", [D_, Bc], F16, isOutput=False)
    # interleaved fp8 x: [k, p, 0, b] = x_lo*2^14, [k, p, 1, b] = x*2^5
    xcomb = nc.declare_dram_parameter("xcomb", [KT, 128, 2, Bc], FP8, isOutput=False)
    # weight slabs pre-arranged on host: w_pre[m, p, k*128 + c] = w1[k*128 + p, m*128 + c]
    twh = nc.declare_dram_parameter("twh", [MT, 128, KT, 128], F16, isOutput=False)
    # interleaved fp8 t_w1: [m, p, k, 0, c] = w_hi*2^4, [m, p, k, 1, c] = w_lo*2^13
    wcomb = nc.declare_dram_parameter("wcomb", [MT, 128, KT, 2, 128], FP8, isOutput=False)
    fwh = nc.declare_dram_parameter("fwh", [MT, 128, KT, 128], F16, isOutput=False)
    # biases b1: [128, MT] with b1s[p, m] = b1[m*128 + p]
    tb1 = nc.declare_dram_parameter("tb1", [128, MT], F32, isOutput=False)
    fb1 = nc.declare_dram_parameter("fb1", [128, MT], F32, isOutput=False)
    # w2: [128, MT*2] with w2s[p, 2m:2m+2] = w2[m*128+p, :]
    tw2 = nc.declare_dram_parameter("tw2", [128, MT * 2], F32, isOutput=False)
    fw2 = nc.declare_dram_parameter("fw2", [128, MT * 2], BF16, isOutput=False)
    tb2 = nc.declare_dram_parameter("tb2", [2, 1], F32, isOutput=False)
    fb2 = nc.declare_dram_parameter("fb2", [2, 1], F32, isOutput=False)
    # [1,-1] and [1,1] helper vectors for the routing epilogue
    dvec = nc.declare_dram_parameter("dvec", [2, 1], F32, isOutput=False)
    ones12 = nc.declare_dram_parameter("ones12", [1, 2], F32, isOutput=False)
    out = nc.declare_dram_parameter("out", [2, Bc], F32, isOutput=True)

    DR = mybir.MatmulPerfMode.DoubleRow

    with TileContext(nc) as tc:
        with (
            tc.tile_pool(name="xres", bufs=1) as xpool,
            tc.tile_pool(name="consts", bufs=1) as cpool,
            tc.tile_pool(name="wstream", bufs=2) as wpool,
            tc.tile_pool(name="hbuf", bufs=3) as hpool,
            tc.tile_pool(name="epi", bufs=1) as epool,
            tc.tile_pool(name="psmm", bufs=1, space="PSUM") as pspool,
            tc.tile_pool(name="pslg", bufs=1, space="PSUM") as ps2pool,
        ):
            # ---- resident loads (consts first; x is prefetch-interleaved
            # with the first weight set below) ----
            xh_t = []
            xc_t = []
            tb1_sb = cpool.tile([128, MT], F32, name="tb1sb")
            nc.sync.dma_start(out=tb1_sb[:], in_=tb1[:])
            fb1_sb = cpool.tile([128, MT], F32, name="fb1sb")
            nc.sync.dma_start(out=fb1_sb[:], in_=fb1[:])
            tw2_sb = cpool.tile([128, MT * 2], F32, name="tw2sb")
            nc.sync.dma_start(out=tw2_sb[:], in_=tw2[:])
            fw2_sb = cpool.tile([128, MT * 2], BF16, name="fw2sb")
            nc.sync.dma_start(out=fw2_sb[:], in_=fw2[:])
            tb2_sb = cpool.tile([2, 1], F32, name="tb2sb")
            nc.sync.dma_start(out=tb2_sb[:], in_=tb2[:])
            fb2_sb = cpool.tile([2, 1], F32, name="fb2sb")
            nc.sync.dma_start(out=fb2_sb[:], in_=fb2[:])
            dvec_sb = cpool.tile([2, 1], F32, name="dvecsb")
            nc.sync.dma_start(out=dvec_sb[:], in_=dvec[:])
            ones_sb = cpool.tile([1, 2], F32, name="onessb")
            nc.sync.dma_start(out=ones_sb[:], in_=ones12[:])

            # Startup DMA order matters (issue order = queue order): the
            # first matmul needs twh[0] + xh[0], so those go first, then the
            # rest of the m=0 set, then the m=1 set (so the m=1 boundary
            # never waits behind the 16MB resident-x stream), then bulk x.
            wsets = {}

            def _wset(m):
                twh_m = wpool.tile([128, KT, 128], F16, name="twhm", tag="twh")
                nc.sync.dma_start(out=twh_m[:], in_=twh[m])
                wc_m = wpool.tile([128, KT, 2, 128], FP8, name="wcm", tag="wcomb")
                nc.sync.dma_start(out=wc_m[:], in_=wcomb[m])
                fwh_m = wpool.tile([128, KT, 128], F16, name="fwhm", tag="fwh")
                nc.sync.dma_start(out=fwh_m[:], in_=fwh[m])
                wsets[m] = (twh_m, wc_m, fwh_m)

            def _xload(k):
                th = xpool.tile([128, Bc], F16, name=f"xh{k}")
                nc.sync.dma_start(out=th[:], in_=xhi[k * 128:(k + 1) * 128, :])
                xh_t.append(th)
                tcb = xpool.tile([128, 2, Bc], FP8, name=f"xc{k}")
                nc.sync.dma_start(out=tcb[:], in_=xcomb[k])
                xc_t.append(tcb)

            _wset(0)
            _xload(0)
            _wset(1)
            for k in range(1, KT):
                _xload(k)

            NS = [(n * 512, (n + 1) * 512) for n in range(NT)]
            ps2_t = [ps2pool.tile([2, 512], F32, name=f"ps2t{n}", tag=f"ps2t{n}")
                     for n in range(NT)]
            ps2_f = [ps2pool.tile([2, 512], F32, name=f"ps2f{n}", tag=f"ps2f{n}")
                     for n in range(NT)]
            for m in range(MT):
                if m in wsets:
                    twh_m, wc_m, fwh_m = wsets.pop(m)
                else:
                    _wset(m)
                    twh_m, wc_m, fwh_m = wsets.pop(m)

                tA = [pspool.tile([128, 512], F32, name=f"tA{n}", tag=f"tA{n}")
                      for n in range(NT)]
                fA = [pspool.tile([128, 512], F32, name=f"fA{n}", tag=f"fA{n}")
                      for n in range(NT)]
                # t-expert main pass (fp16, products at 2^18); one stationary
                # load serves both n tiles
                for k in range(KT):
                    for n, (c0, c1) in enumerate(NS):
                        nc.tensor.matmul(
                            tA[n][:], twh_m[:, k], xh_t[k][:, c0:c1],
                            start=(k == 0), stop=False,
                        )
                # t-expert fp8 DoubleRow correction (both terms, scale 2^18)
                for k in range(KT):
                    for n, (c0, c1) in enumerate(NS):
                        nc.tensor.matmul(
                            tA[n][:], wc_m[:, k], xc_t[k][:, :, c0:c1],
                            start=False, stop=(k == KT - 1), perf_mode=DR,
                        )
                # f-expert (fp16, products at 2^10)
                for k in range(KT):
                    for n, (c0, c1) in enumerate(NS):
                        nc.tensor.matmul(
                            fA[n][:], fwh_m[:, k], xh_t[k][:, c0:c1],
                            start=(k == 0), stop=(k == KT - 1),
                        )
                # relu(tA) runs on ACT while the f matmuls stream, so the
                # second-layer matmuls at the end of the iteration don't stall
                ht = []
                for n in range(NT):
                    t_ = hpool.tile([128, 512], F32, name=f"ht{n}", tag=f"ht{n}")
                    nc.scalar.activation(
                        t_[:], tA[n][:], mybir.ActivationFunctionType.Relu,
                        bias=tb1_sb[:, m:m + 1],
                    )
                    ht.append(t_)
                hf = []
                for n in range(NT):
                    f_ = hpool.tile([128, 512], BF16, name=f"hf{n}", tag=f"hf{n}")
                    nc.scalar.activation(
                        f_[:], fA[n][:], mybir.ActivationFunctionType.Relu,
                        bias=fb1_sb[:, m:m + 1],
                    )
                    hf.append(f_)
                for n in range(NT):
                    nc.tensor.matmul(
                        ps2_t[n][:], tw2_sb[:, 2 * m:2 * m + 2], ht[n][:],
                        start=(m == 0), stop=(m == MT - 1),
                    )
                for n in range(NT):
                    nc.tensor.matmul(
                        ps2_f[n][:], fw2_sb[:, 2 * m:2 * m + 2], hf[n][:],
                        start=(m == 0), stop=(m == MT - 1),
                    )

            # ---------------- routing epilogue ----------------
            # PSUM psd/psb borrow the now-dead accumulator rings (tags tA0/fA0)
            for n, (c0, c1) in enumerate(NS):
                tl = epool.tile([2, 512], F32, name=f"tlg{n}", tag="tlg")
                nc.vector.tensor_scalar_add(tl[:], ps2_t[n][:], tb2_sb[:, 0:1])
                fl = epool.tile([2, 512], F32, name=f"flg{n}", tag="flg")
                nc.vector.tensor_scalar_add(fl[:], ps2_f[n][:], fb2_sb[:, 0:1])
                # d = t0 - t1 via PE: [2,1]^T @ [2,512] -> [1,512]
                psd_full = pspool.tile([128, 512], F32, name=f"psdf{n}", tag="tA0")
                ps_d = psd_full[0:1, :]
                nc.tensor.matmul(ps_d, dvec_sb[:], tl[:], start=True, stop=True)
                ad = epool.tile([1, 512], F32, name=f"ad{n}", tag="ad")
                nc.scalar.activation(ad[:], ps_d, mybir.ActivationFunctionType.Abs)
                # broadcast |d| to both partitions: [1,2]^T @ [1,512] -> [2,512]
                psb_full = pspool.tile([128, 512], F32, name=f"psbf{n}", tag="fA0")
                ps_b = psb_full[0:2, :]
                nc.tensor.matmul(ps_b, ones_sb[:], ad[:], start=True, stop=True)
                mk = epool.tile([2, 512], mybir.dt.uint8, name=f"mk{n}", tag="mk")
                nc.vector.tensor_scalar(
                    mk[:], ps_b, LN4, None, op0=mybir.AluOpType.is_lt
                )
                ob = epool.tile([2, 512], F32, name=f"ob{n}", tag="ob")
                nc.vector.select(ob[:], mk[:], fl[:], tl[:])
                nc.sync.dma_start(out=out[:, c0:c1], in_=ob[:])

    _prune_weight_dma_waits(nc)
    _fix_wait_overflow(nc)
    return nc


def _fix_wait_overflow(nc):
    """Walrus engine/DMA instructions accept at most 2 sync commands
    (waits + updates) total, but InstDrain accepts many waits. For any
    instruction exceeding the budget, hoist the extra waits onto an
    InstDrain inserted just before it on the same engine queue — the
    queue is in-order, so the following instruction only issues after
    the drain's waits are satisfied."""
    import concourse.mybir as _mybir

    seq = 0
    for bb in nc.m.functions[0].blocks:
        out_list = []
        for ins in bb.instructions:
            si = getattr(ins, "sync_info", None)
            if si is not None and type(ins).__name__ == "InstDrain":
                # split over-budget drains into a chain of <=2-wait drains
                waits = list(si.on_wait or [])
                if len(waits) + len(si.on_update or []) > 2:
                    while len(waits) > 1:
                        chunk, waits = waits[:1], waits[1:]
                        dr = _mybir.InstDrain(
                            name=f"WOF-{seq}", engine=ins.engine, ins=[], outs=[],
                            sync_info=_mybir.SyncInfo(on_wait=chunk, on_update=[]),
                        )
                        seq += 1
                        out_list.append(dr)
                    ins.sync_info = _mybir.SyncInfo(
                        on_wait=waits, on_update=si.on_update
                    )
                out_list.append(ins)
                continue
            if (
                si is not None
                and len(si.on_wait or []) + len(si.on_update or []) > 2
            ):
                n_upd = len(si.on_update or [])
                keep = max(0, 2 - n_upd - 1) + 1 if n_upd <= 1 else 0
                keep = min(keep, len(si.on_wait))
                extras = list(si.on_wait[keep:])
                if extras:
                    for i in range(0, len(extras), 1):
                        dr = _mybir.InstDrain(
                            name=f"WOF-{seq}",
                            engine=ins.engine,
                            ins=[],
                            outs=[],
                            sync_info=_mybir.SyncInfo(
                                on_wait=extras[i:i + 1], on_update=[]
                            ),
                        )
                        seq += 1
                        out_list.append(dr)
                    ins.sync_info = _mybir.SyncInfo(
                        on_wait=list(si.on_wait[:keep]), on_update=si.on_update
                    )
            out_list.append(ins)
        bb.instructions[:] = out_list


def _prune_weight_dma_waits(nc):
    """Walrus allows a single sem wait per DMA instruction, but Tile emits
    [PE-WAR, DMA-lane-WAW] pairs on the recycled weight-stream slots. The
    DMA-lane waits are redundant: the kept PE wait covers the tick of the
    last matmul that read the old slot contents, and that matmul itself
    waited on the old slot's DMA before reading. Drop the DMA-lane waits.

    Only applied to DMAs sourced from the weight-stream DRAM tensors, whose
    slots are written by DMA and read exclusively by the PE."""
    import concourse.mybir as _mybir

    wsrc = {"twh", "wcomb", "fwh"}
    for bb in nc.m.functions[0].blocks:
        for ins in bb.instructions:
            if type(ins).__name__ != "InstDMACopy":
                continue
            si = ins.sync_info
            if si is None or len(si.on_wait or []) <= 1:
                continue
            src = getattr(ins.ins[0], "memref", None)
            assert src in wsrc, f"unexpected multi-wait DMA from {src}"
            pe = [w for w in si.on_wait if w.ant_name.startswith("PE")]
            dropped = [w for w in si.on_wait if not w.ant_name.startswith("PE")]
            assert len(pe) == 1 and all(
                w.ant_name.startswith(("DMAHW", "DMASW")) for w in dropped
            ), f"unexpected wait mix on {ins.name}: " \
               f"{[w.ant_name for w in si.on_wait]}"
            ins.sync_info = _mybir.SyncInfo(on_wait=pe, on_update=si.on_update)


def _bf(a):
    return a.astype(ml_dtypes.bfloat16)


def _prep_w1(w, KT, MT):
    """[D,H] -> [MT, 128, KT*128] with w_pre[m,p,k*128+c] = w[k*128+p, m*128+c]"""
    D_, H_ = w.shape
    return np.ascontiguousarray(
        w.reshape(KT, 128, MT, 128).transpose(2, 1, 0, 3).reshape(MT, 128, KT * 128)
    )


def _prep_b1(b, MT):
    return np.ascontiguousarray(b.reshape(MT, 128).T)


def _prep_w2(w, MT):
    return np.ascontiguousarray(
        w.reshape(MT, 128, 2).transpose(1, 0, 2).reshape(128, MT * 2)
    )


_CACHED = {}


def _get_nc(Bc, D_, H_):
    key = (Bc, D_, H_)
    if key not in _CACHED:
        _CACHED[key] = build_nc(Bc, D_, H_)
    return _CACHED[key]


LAST_EXEC_TIME_NS = None


def kernel(x, t_w1, t_b1, t_w2, t_b2, f_w1, f_b1, f_w2, f_b2, _trace=False):
    global LAST_EXEC_TIME_NS
    x = np.asarray(x, dtype=np.float32)
    Bfull, D_ = x.shape
    H_ = t_w1.shape[1]
    Bc = Bfull // N_CORES
    KT, MT = D_ // 128, H_ // 128

    t_w1 = np.asarray(t_w1, dtype=np.float32)
    f_w1 = np.asarray(f_w1, dtype=np.float32)

    # fp16 main-pass weights at scale 2^18; whi/wlo split derives from the cast
    t_hi_scaled = (t_w1 * 2.0**SCALE_BITS).astype(np.float16).astype(np.float32)
    t_hi32 = t_hi_scaled * 2.0**-SCALE_BITS
    t_lo32 = t_w1 - t_hi32
    twh = _prep_w1(t_hi_scaled, KT, MT).astype(np.float16)
    # fp8 DoubleRow correction weights [MT, 128, KT, 2, 128]
    whi8 = _prep_w1(t_hi32 * 2.0**S_WHI, KT, MT).reshape(MT, 128, KT, 128)
    wlo8 = _prep_w1(t_lo32 * 2.0**S_WLO, KT, MT).reshape(MT, 128, KT, 128)
    wcomb = np.stack([whi8, wlo8], axis=3).astype(NP_FP8)
    fwh = _prep_w1(f_w1 * 2.0**F_SCALE, KT, MT).astype(np.float16)

    tb1s = _prep_b1(np.asarray(t_b1, np.float32) * 2.0**SCALE_BITS, MT)
    fb1s = _prep_b1(np.asarray(f_b1, np.float32) * 2.0**F_SCALE, MT)
    tw2s = _prep_w2(np.asarray(t_w2, np.float32) * 2.0**-SCALE_BITS, MT)
    fw2s = _prep_w2(
        np.asarray(f_w2, np.float32) * 2.0**-F_SCALE, MT
    ).astype(ml_dtypes.bfloat16)
    tb2s = np.asarray(t_b2, np.float32).reshape(2, 1)
    fb2s = np.asarray(f_b2, np.float32).reshape(2, 1)

    shared = dict(
        twh=twh, wcomb=wcomb, fwh=fwh, tb1=tb1s, fb1=fb1s,
        tw2=tw2s, fw2=fw2s, tb2=tb2s, fb2=fb2s,
        dvec=np.array([[1.0], [-1.0]], np.float32),
        ones12=np.array([[1.0, 1.0]], np.float32),
    )
    in_maps = []
    for c in range(N_CORES):
        xc = np.ascontiguousarray(x[c * Bc:(c + 1) * Bc].T)  # [D, Bc]
        xc16 = xc.astype(np.float16)
        xc_lo32 = xc - xc16.astype(np.float32)
        xlo8 = (xc_lo32 * 2.0**S_XLO).reshape(KT, 128, Bc)
        xfu8 = (xc * 2.0**S_XHI).reshape(KT, 128, Bc)
        xcomb = np.stack([xlo8, xfu8], axis=2).astype(NP_FP8)
        in_maps.append(dict(shared, xhi=xc16, xcomb=xcomb))

    nc = _get_nc(Bc, D_, H_)
    res = run_bass_kernel_spmd(nc, in_maps, list(range(N_CORES)), trace=_trace)
    LAST_EXEC_TIME_NS = res.exec_time_ns
    outT = np.concatenate([res.results[c]["out"] for c in range(N_CORES)], axis=1)
    return np.ascontiguousarray(outT.T.astype(np.float32))
